# revision 15
# baseline (speedup 1.0000x reference)
"""DeepSeekMoE Trainium2 kernel: expert-parallel across 8 NeuronCores.

Strategy:
  - Host computes routing (3 small sigmoid routers + top-k) and performs the
    token all-to-all: for each expert e (= core e), gather the tokens that
    chose e in their top-2, grouped by their top-1 sub-expert, padded to a
    static capacity.  Tokens ship transposed ([H, tokens]) so the two matmul
    chains need no on-chip transposes:
        H1^T = W1^T @ X^T   (gelu + b1 fused on ScalarE)
        Y^T  = W2^T @ H1^T
    Weights are the stationary PE operand in their natural [in, out] layout.
  - The shared-expert path is data-parallel: core c processes tokens
    [c*256, (c+1)*256), grouped by top-1 shared sub-expert.  It runs FIRST
    (its weights are small) to warm the PE while expert weights stream in.
  - Every device tensor is host-prepacked into its SBUF image ([128 x free],
    contiguous per partition) so each load/store is one large-chunk 2D DMA:
    SWDGE descriptor-generation cost (~0.6-1.7us per fragmented dma_start)
    was the dominant serial cost otherwise.
  - Host applies the second bias + sigmoid gates and scatter-adds back.
Only the routing/dispatch (<1% of FLOPs) runs on host; both FFN chains for
every selected (expert, sub-expert) combo run on device in fp16 (PE runs
fp16 at 4x the fp32 rate; fp32 PSUM accumulation keeps rel-err ~3e-4).
"""

import sys

sys.path.insert(0, "/opt/trn_rl_repo")

import numpy as np

import concourse.bass as bass  # noqa: F401  (registers AP machinery)
import concourse.mybir as mybir
from concourse import bacc
from concourse.tile import TileContext
from concourse.bass_utils import run_bass_kernel_spmd

N, H, E, S = 2048, 512, 8, 4
F_SH = 128
P = 128
KT = H // P  # 4 k-tiles over the hidden dim
NCORES = 8
N_PER_CORE = N // NCORES

MM_DT = mybir.dt.float16
OUT_DT = mybir.dt.float16

_compiled_cache: dict = {}
_last_res = None


def _roundup(v: int, m: int) -> int:
    return ((v + m - 1) // m) * m


def _np_dt(dt):
    return mybir.dt.np(dt)


def _build_module(capR: int, capS: int):
    """Build + compile the SPMD per-core Bass module."""
    GR = S * capR
    GS = S * capS

    nc = bacc.Bacc("TRN2", target_bir_lowering=False, debug=False)
    dt = MM_DT

    # All inputs/outputs are SBUF images: [128, free], contiguous.
    xrp = nc.dram_tensor("xrp", [P, KT * GR], dt, kind="ExternalInput").ap()
    xsp = nc.dram_tensor("xsp", [P, KT * GS], dt, kind="ExternalInput").ap()
    w1p = nc.dram_tensor("w1p", [P, S * KT * H], dt, kind="ExternalInput").ap()
    w2p = nc.dram_tensor("w2p", [P, S * KT * H], dt, kind="ExternalInput").ap()
    sw1p = nc.dram_tensor("sw1p", [P, S * KT * F_SH], dt, kind="ExternalInput").ap()
    sw2p = nc.dram_tensor("sw2p", [P, S * H], dt, kind="ExternalInput").ap()
    b1p = nc.dram_tensor("b1p", [P, S * KT], mybir.dt.float32, kind="ExternalInput").ap()
    sb1p = nc.dram_tensor("sb1p", [P, S], mybir.dt.float32, kind="ExternalInput").ap()

    yrp = nc.dram_tensor("yrp", [P, S * KT * capR], OUT_DT, kind="ExternalOutput").ap()
    ysp = nc.dram_tensor("ysp", [P, S * KT * capS], OUT_DT, kind="ExternalOutput").ap()

    GELU = mybir.ActivationFunctionType.Gelu

    with TileContext(nc) as tc:
        with (
            tc.tile_pool(name="weights", bufs=1) as wpool,
            tc.tile_pool(name="acts", bufs=1) as apool,
            tc.tile_pool(name="outs", bufs=2) as opool,
            tc.tile_pool(name="psum", bufs=4, space="PSUM") as ppool,
        ):
            # ---- loads.  gpsimd queue: shared path + activations (consumed
            # first); sync queue: the 8 big expert-weight slices, s-major.
            sb1_sb = wpool.tile([P, S], mybir.dt.float32, tag="sb1")
            nc.gpsimd.dma_start(out=sb1_sb[:], in_=sb1p[:])
            xs_sb = apool.tile([P, KT * GS], dt, tag="xs")
            nc.gpsimd.dma_start(out=xs_sb[:], in_=xsp[:])
            sw1_sb = wpool.tile([P, S * KT * F_SH], dt, tag="sw1")
            nc.gpsimd.dma_start(out=sw1_sb[:], in_=sw1p[:])
            sw2_sb = wpool.tile([P, S * H], dt, tag="sw2")
            nc.gpsimd.dma_start(out=sw2_sb[:], in_=sw2p[:])
            b1_sb = wpool.tile([P, S * KT], mybir.dt.float32, tag="b1")
            nc.gpsimd.dma_start(out=b1_sb[:], in_=b1p[:])
            xr_sb = apool.tile([P, KT * GR], dt, tag="xr")
            nc.gpsimd.dma_start(out=xr_sb[:], in_=xrp[:])

            w1_sb = {}
            w2_sb = {}
            for s in range(S):
                t = wpool.tile([P, KT * H], dt, tag=f"w1_{s}")
                nc.sync.dma_start(out=t[:], in_=w1p[:, s * KT * H : (s + 1) * KT * H])
                w1_sb[s] = t
                t = wpool.tile([P, KT * H], dt, tag=f"w2_{s}")
                nc.sync.dma_start(out=t[:], in_=w2p[:, s * KT * H : (s + 1) * KT * H])
                w2_sb[s] = t

            # ---- shared path (small; runs first to warm the PE) ------------
            for s in range(S):
                ps = ppool.tile([P, capS], mybir.dt.float32, tag="ps1")
                for k in range(KT):
                    nc.tensor.matmul(
                        ps,
                        sw1_sb[:, (s * KT + k) * F_SH : (s * KT + k + 1) * F_SH],
                        xs_sb[:, k * GS + s * capS : k * GS + s * capS + capS],
                        start=(k == 0),
                        stop=(k == KT - 1),
                    )
                hs = apool.tile([P, capS], dt, tag="hs")
                nc.scalar.activation(hs[:], ps[:], GELU, bias=sb1_sb[:, s : s + 1])
                o = opool.tile([P, KT * capS], OUT_DT, tag="ys_o")
                for hidx in range(KT):
                    ps2 = ppool.tile([P, capS], mybir.dt.float32, tag="ps2")
                    nc.tensor.matmul(
                        ps2,
                        sw2_sb[:, s * H + hidx * P : s * H + (hidx + 1) * P],
                        hs[:],
                        start=True,
                        stop=True,
                    )
                    nc.vector.tensor_copy(o[:, hidx * capS : (hidx + 1) * capS], ps2[:])
                nc.gpsimd.dma_start(
                    out=ysp[:, s * KT * capS : (s + 1) * KT * capS], in_=o[:]
                )

            # ---- routed path -----------------------------------------------
            for s in range(S):
                h1_sb = []
                for f in range(KT):
                    ps = ppool.tile([P, capR], mybir.dt.float32, tag="ps1")
                    for k in range(KT):
                        nc.tensor.matmul(
                            ps,
                            w1_sb[s][:, k * H + f * P : k * H + (f + 1) * P],
                            xr_sb[:, k * GR + s * capR : k * GR + s * capR + capR],
                            start=(k == 0),
                            stop=(k == KT - 1),
                        )
                    h1 = apool.tile([P, capR], dt, tag=f"h1_{f}")
                    nc.scalar.activation(
                        h1[:], ps[:], GELU, bias=b1_sb[:, s * KT + f : s * KT + f + 1]
                    )
                    h1_sb.append(h1)
                o = opool.tile([P, KT * capR], OUT_DT, tag="yr_o")
                for hidx in range(KT):
                    ps = ppool.tile([P, capR], mybir.dt.float32, tag="ps2")
                    for f in range(KT):
                        nc.tensor.matmul(
                            ps,
                            w2_sb[s][:, f * H + hidx * P : f * H + (hidx + 1) * P],
                            h1_sb[f][:],
                            start=(f == 0),
                            stop=(f == KT - 1),
                        )
                    nc.vector.tensor_copy(o[:, hidx * capR : (hidx + 1) * capR], ps[:])
                nc.scalar.dma_start(
                    out=yrp[:, s * KT * capR : (s + 1) * KT * capR], in_=o[:]
                )

    nc.compile()
    return nc


def _sigmoid(v):
    out = np.empty_like(v)
    np.negative(np.abs(v), out=out)
    np.exp(out, out=out)
    pos = v >= 0
    out_pos = 1.0 / (1.0 + out)
    out_neg = out / (1.0 + out)
    return np.where(pos, out_pos, out_neg)


def _pack_weight(w, np_dt):
    """[S, R*P, C] -> SBUF image [P, S*R*C] (R = rows/P k-tiles)."""
    s, rp, c = w.shape
    r = rp // P
    return np.ascontiguousarray(
        w.reshape(s, r, P, c).transpose(2, 0, 1, 3).reshape(P, s * r * c)
    ).astype(np_dt)


def _pack_acts(xt, np_dt):
    """[H, G] -> SBUF image [P, KT*G]."""
    g = xt.shape[1]
    return np.ascontiguousarray(
        xt.reshape(KT, P, g).transpose(1, 0, 2).reshape(P, KT * g)
    ).astype(np_dt)


def kernel(**inputs) -> np.ndarray:
    x = np.ascontiguousarray(np.asarray(inputs["x"], dtype=np.float32))
    shared_W1 = np.asarray(inputs["shared_W1"], dtype=np.float32)
    shared_b1 = np.asarray(inputs["shared_b1"], dtype=np.float32)
    shared_W2 = np.asarray(inputs["shared_W2"], dtype=np.float32)
    shared_b2 = np.asarray(inputs["shared_b2"], dtype=np.float32)
    shared_router_W = np.asarray(inputs["shared_router_W"], dtype=np.float32)
    shared_router_b = np.asarray(inputs["shared_router_b"], dtype=np.float32)
    expert_W1 = np.asarray(inputs["expert_W1"], dtype=np.float32)
    expert_b1 = np.asarray(inputs["expert_b1"], dtype=np.float32)
    expert_W2 = np.asarray(inputs["expert_W2"], dtype=np.float32)
    expert_b2 = np.asarray(inputs["expert_b2"], dtype=np.float32)
    router_W = np.asarray(inputs["router_W"], dtype=np.float32)
    router_b = np.asarray(inputs["router_b"], dtype=np.float32)
    sub_router_W = np.asarray(inputs["sub_router_W"], dtype=np.float32)
    sub_router_b = np.asarray(inputs["sub_router_b"], dtype=np.float32)
    expert_bias = np.asarray(inputs["expert_bias"], dtype=np.float32)
    sub_expert_bias = np.asarray(inputs["sub_expert_bias"], dtype=np.float32)

    n = x.shape[0]
    assert x.shape == (N, H)

    # ---- host routing (matches reference's router math) --------------------
    sp = _sigmoid(x @ shared_router_W + shared_router_b + sub_expert_bias)  # [n,S]
    si = np.argmax(sp, axis=1)  # top-1 shared sub-expert
    sw = sp[np.arange(n), si]

    rp = _sigmoid(x @ router_W + router_b + expert_bias)  # [n,E]
    ei = np.argsort(-rp, axis=1, kind="stable")[:, :2]  # top-2 experts
    ew = np.take_along_axis(rp, ei, axis=1)  # [n,2]

    subp = _sigmoid(x @ sub_router_W + sub_router_b + sub_expert_bias)
    ssi = np.argmax(subp, axis=1)  # top-1 routed sub-expert (gate NOT applied)

    # ---- dispatch: group routed slots by (expert, sub-expert) --------------
    flat_tok = np.repeat(np.arange(n), 2)
    flat_e = ei.reshape(-1)
    flat_gate = ew.reshape(-1)
    flat_s = ssi[flat_tok]
    group = flat_e * S + flat_s
    counts = np.bincount(group, minlength=E * S)
    capR = max(64, _roundup(int(counts.max()), 16))

    sort_idx = np.argsort(group, kind="stable")
    g_tok = flat_tok[sort_idx]
    g_gate = flat_gate[sort_idx]
    g_off = np.concatenate([[0], np.cumsum(counts)])

    # shared groups: per core slice of 256 tokens, grouped by si
    capS_counts = []
    for c in range(NCORES):
        sl = si[c * N_PER_CORE : (c + 1) * N_PER_CORE]
        capS_counts.append(np.bincount(sl, minlength=S))
    capS_counts = np.stack(capS_counts)  # [NCORES, S]
    capS = max(32, _roundup(int(capS_counts.max()), 16))

    np_dt = _np_dt(MM_DT)
    xT = np.ascontiguousarray(x.T)  # [H, N] fp32; cast after gather

    GR, GS = S * capR, S * capS
    in_maps = []
    tok_es = {}
    stok_cs = {}
    for c in range(NCORES):
        e = c
        xr_host = np.zeros((H, GR), dtype=np.float32)
        for s in range(S):
            g = e * S + s
            toks = g_tok[g_off[g] : g_off[g + 1]]
            tok_es[e, s] = (toks, g_gate[g_off[g] : g_off[g + 1]])
            xr_host[:, s * capR : s * capR + len(toks)] = xT[:, toks]

        xs_host = np.zeros((H, GS), dtype=np.float32)
        base = c * N_PER_CORE
        sl = si[base : base + N_PER_CORE]
        for s in range(S):
            toks = base + np.nonzero(sl == s)[0]
            stok_cs[c, s] = toks
            xs_host[:, s * capS : s * capS + len(toks)] = xT[:, toks]

        b1p = np.ascontiguousarray(
            expert_b1[e].reshape(S, KT, P).transpose(2, 0, 1).reshape(P, S * KT)
        ).astype(np.float32)
        sb1p = np.ascontiguousarray(shared_b1.T).astype(np.float32)

        in_maps.append(
            {
                "xrp": _pack_acts(xr_host, np_dt),
                "xsp": _pack_acts(xs_host, np_dt),
                "w1p": _pack_weight(expert_W1[e], np_dt),
                "w2p": _pack_weight(expert_W2[e], np_dt),
                "sw1p": _pack_weight(shared_W1, np_dt),
                "sw2p": _pack_weight(shared_W2, np_dt),
                "b1p": b1p,
                "sb1p": sb1p,
            }
        )

    key = (capR, capS, MM_DT)
    nc = _compiled_cache.get(key)
    if nc is None:
        import time as _time

        _t = _time.time()
        nc = _build_module(capR, capS)
        print(f"[kernel] built module capR={capR} capS={capS} "
              f"in {_time.time() - _t:.1f}s", flush=True)
        _compiled_cache[key] = nc

    res = run_bass_kernel_spmd(nc, in_maps, core_ids=list(range(NCORES)))
    global _last_res
    _last_res = res

    # ---- host combine ------------------------------------------------------
    out = np.zeros((N, H), dtype=np.float32)
    for c in range(NCORES):
        e = c
        # unpack SBUF images: [P, S*KT*cap] -> per (s): [H, cap]
        yr_out = (
            res.results[c]["yrp"].reshape(P, S, KT, capR).transpose(1, 2, 0, 3)
        ).reshape(S, H, capR)
        ys_out = (
            res.results[c]["ysp"].reshape(P, S, KT, capS).transpose(1, 2, 0, 3)
        ).reshape(S, H, capS)
        for s in range(S):
            toks, gates = tok_es[e, s]
            if len(toks):
                ycols = yr_out[s, :, : len(toks)].T.astype(np.float32)  # [cnt, H]
                out[toks] += gates[:, None] * (ycols + expert_b2[e, s])
            stoks = stok_cs[c, s]
            if len(stoks):
                ycols = ys_out[s, :, : len(stoks)].T.astype(np.float32)
                out[stoks] += sw[stoks, None] * (ycols + shared_b2[s])

    return out


# revision 18
# speedup vs baseline: 1.0816x; 1.0816x over previous
"""DeepSeekMoE Trainium2 kernel: expert-parallel across 8 NeuronCores.

Strategy:
  - Host computes routing (3 small sigmoid routers + top-k) and performs the
    token all-to-all: for each expert e (= core e), gather the tokens that
    chose e in their top-2, grouped by their top-1 sub-expert, padded to a
    static capacity.  Tokens ship transposed ([H, tokens]) so the two matmul
    chains need no on-chip transposes:
        H1^T = W1^T @ X^T   (gelu + b1 fused on ScalarE)
        Y^T  = W2^T @ H1^T
    Weights are the stationary PE operand in their natural [in, out] layout.
  - The shared-expert path is data-parallel: core c processes tokens
    [c*256, (c+1)*256), grouped by top-1 shared sub-expert.  It runs FIRST
    (its weights are small) to warm the PE while expert weights stream in.
  - Every device tensor is host-prepacked into its SBUF image ([128 x free],
    contiguous per partition) so each load/store is one large-chunk 2D DMA:
    SWDGE descriptor-generation cost (~0.6-1.7us per fragmented dma_start)
    was the dominant serial cost otherwise.
  - Host applies the second bias + sigmoid gates and scatter-adds back.
Only the routing/dispatch (<1% of FLOPs) runs on host; both FFN chains for
every selected (expert, sub-expert) combo run on device in fp16 (PE runs
fp16 at 4x the fp32 rate; fp32 PSUM accumulation keeps rel-err ~3e-4).
"""

import sys

sys.path.insert(0, "/opt/trn_rl_repo")

import numpy as np

import concourse.bass as bass  # noqa: F401  (registers AP machinery)
import concourse.mybir as mybir
from concourse import bacc
from concourse.tile import TileContext
from concourse.bass_utils import run_bass_kernel_spmd

# If tracing is requested (BASS_TRACE=1) bass_utils imports
# antenv.axon_hooks, which this image's antenv package lacks — install a
# shim wired to the ctypes NTFF hook so tracing degrades gracefully.
try:
    import antenv.axon_hooks  # noqa: F401
except ImportError:
    try:
        import types as _types

        import antenv as _antenv

        _hooks = _types.ModuleType("antenv.axon_hooks")
        _hook_box = [None]
        _hooks.set_axon_ntff_profile_hook = lambda h: _hook_box.__setitem__(0, h)
        _hooks.get_axon_ntff_profile_hook = lambda: _hook_box[0]
        sys.modules["antenv.axon_hooks"] = _hooks
        _antenv.axon_hooks = _hooks
        try:
            from trn_agent_boot.trn_boot import _ntff_profile_via_ctypes

            _hooks.set_axon_ntff_profile_hook(
                _ntff_profile_via_ctypes("/opt/axon/libaxon_pjrt.so")
            )
        except Exception:
            pass
    except Exception:
        pass

N, H, E, S = 2048, 512, 8, 4
F_SH = 128
P = 128
KT = H // P  # 4 k-tiles over the hidden dim
NCORES = 8
N_PER_CORE = N // NCORES

MM_DT = mybir.dt.float16
OUT_DT = mybir.dt.float16

_compiled_cache: dict = {}
_last_res = None


def _roundup(v: int, m: int) -> int:
    return ((v + m - 1) // m) * m


def _np_dt(dt):
    return mybir.dt.np(dt)


def _build_module(capR: int, capS: int):
    """Build + compile the SPMD per-core Bass module."""
    GR = S * capR
    GS = S * capS

    nc = bacc.Bacc("TRN2", target_bir_lowering=False, debug=False)
    dt = MM_DT

    # All inputs/outputs are SBUF images: [128, free], contiguous.
    xrp = nc.dram_tensor("xrp", [P, KT * GR], dt, kind="ExternalInput").ap()
    xsp = nc.dram_tensor("xsp", [P, KT * GS], dt, kind="ExternalInput").ap()
    w1p = nc.dram_tensor("w1p", [P, S * KT * H], dt, kind="ExternalInput").ap()
    w2p = nc.dram_tensor("w2p", [P, S * KT * H], dt, kind="ExternalInput").ap()
    sw1p = nc.dram_tensor("sw1p", [P, S * KT * F_SH], dt, kind="ExternalInput").ap()
    sw2p = nc.dram_tensor("sw2p", [P, S * H], dt, kind="ExternalInput").ap()
    b1p = nc.dram_tensor("b1p", [P, S * KT], mybir.dt.float32, kind="ExternalInput").ap()
    sb1p = nc.dram_tensor("sb1p", [P, S], mybir.dt.float32, kind="ExternalInput").ap()

    yrp = nc.dram_tensor("yrp", [P, S * KT * capR], OUT_DT, kind="ExternalOutput").ap()
    ysp = nc.dram_tensor("ysp", [P, S * KT * capS], OUT_DT, kind="ExternalOutput").ap()

    GELU = mybir.ActivationFunctionType.Gelu
    # pair two f/h tiles per PSUM bank when they fit (halves bank pressure
    # so two sub-expert groups can be in flight)
    fpgR = 2 if 2 * capR * 4 <= 2048 else 1
    fpgS = 2 if 2 * capS * 4 <= 2048 else 1

    with TileContext(nc) as tc:
        with (
            tc.tile_pool(name="weights", bufs=1) as wpool,
            tc.tile_pool(name="acts", bufs=1) as apool,
            tc.tile_pool(name="h1s", bufs=2) as hpool,
            tc.tile_pool(name="outs", bufs=2) as opool,
            tc.tile_pool(name="psum1", bufs=4, space="PSUM") as ppool1,
            tc.tile_pool(name="psum2", bufs=3, space="PSUM") as ppool2,
            tc.tile_pool(name="psumw", bufs=1, space="PSUM") as ppoolw,
        ):
            # ---- loads: ALL on the sync HWDGE queue in consumption order so
            # the small shared-path tensors beat the 4MB weight stream to the
            # shared DMA queues.
            xs_sb = apool.tile([P, KT * GS], dt, tag="xs")
            nc.sync.dma_start(out=xs_sb[:], in_=xsp[:])
            sw1_sb = wpool.tile([P, S * KT * F_SH], dt, tag="sw1")
            nc.sync.dma_start(out=sw1_sb[:], in_=sw1p[:])
            sw2_sb = wpool.tile([P, S * H], dt, tag="sw2")
            nc.sync.dma_start(out=sw2_sb[:], in_=sw2p[:])
            sb1_sb = wpool.tile([P, S], mybir.dt.float32, tag="sb1")
            nc.sync.dma_start(out=sb1_sb[:], in_=sb1p[:])
            b1_sb = wpool.tile([P, S * KT], mybir.dt.float32, tag="b1")
            nc.sync.dma_start(out=b1_sb[:], in_=b1p[:])
            xr_sb = apool.tile([P, KT * GR], dt, tag="xr")
            nc.sync.dma_start(out=xr_sb[:], in_=xrp[:])
            w1_sb = {}
            w2_sb = {}
            for s in range(S):
                t = wpool.tile([P, KT * H], dt, tag=f"w1_{s}")
                nc.sync.dma_start(out=t[:], in_=w1p[:, s * KT * H : (s + 1) * KT * H])
                w1_sb[s] = t
                t = wpool.tile([P, KT * H], dt, tag=f"w2_{s}")
                nc.sync.dma_start(out=t[:], in_=w2p[:, s * KT * H : (s + 1) * KT * H])
                w2_sb[s] = t

            # ---- PE warm-up: ~3.5us of dummy matmuls during the DMA wait
            # flips the HAM clock gate to 2.4GHz before the real work lands.
            zw = wpool.tile([P, 512], dt, tag="warm_in")
            nc.vector.memset(zw[:], 0.0)
            wps = ppoolw.tile([P, 512], mybir.dt.float32, tag="warm_ps")
            for _ in range(8):
                nc.tensor.matmul(wps, zw[:, :P], zw[:], start=True, stop=True)

            # ---- shared path (small; runs first while weights stream in) ---
            sh_ps1 = {}
            for s in range(S):
                g, off = divmod(s, fpgS)
                if off == 0:
                    sh_ps1[g] = ppool1.tile(
                        [P, fpgS * capS], mybir.dt.float32, tag="ps1", name=f"shps1_{g}"
                    )
                for k in range(KT):
                    nc.tensor.matmul(
                        sh_ps1[g][:, off * capS : (off + 1) * capS],
                        sw1_sb[:, (s * KT + k) * F_SH : (s * KT + k + 1) * F_SH],
                        xs_sb[:, k * GS + s * capS : k * GS + s * capS + capS],
                        start=(k == 0),
                        stop=(k == KT - 1),
                    )
            hs_sb = {}
            for s in range(S):
                g, off = divmod(s, fpgS)
                hs = hpool.tile([P, capS], dt, tag=f"hs_{s}")
                nc.scalar.activation(
                    hs[:],
                    sh_ps1[g][:, off * capS : (off + 1) * capS],
                    GELU,
                    bias=sb1_sb[:, s : s + 1],
                )
                hs_sb[s] = hs
            for s in range(S):
                o = opool.tile([P, KT * capS], OUT_DT, tag="ys_o")
                for hg in range(KT // fpgS):
                    ps2 = ppool2.tile([P, fpgS * capS], mybir.dt.float32, tag="ps2")
                    for off in range(fpgS):
                        hidx = hg * fpgS + off
                        nc.tensor.matmul(
                            ps2[:, off * capS : (off + 1) * capS],
                            sw2_sb[:, s * H + hidx * P : s * H + (hidx + 1) * P],
                            hs_sb[s][:],
                            start=True,
                            stop=True,
                        )
                    nc.vector.tensor_copy(
                        o[:, hg * fpgS * capS : (hg + 1) * fpgS * capS], ps2[:]
                    )
                nc.gpsimd.dma_start(
                    out=ysp[:, s * KT * capS : (s + 1) * KT * capS], in_=o[:]
                )

            # ---- routed path, software-pipelined on the PE:
            #   MM1(0), MM1(1), MM2(0), MM1(2), MM2(1), MM1(3), MM2(2), MM2(3)
            # so the gelu latency between a sub-expert's two chains is hidden
            # behind the next sub-expert's first chain.
            h1_tiles = {}

            def emit_mm1(s):
                pairs = {}
                for fg in range(KT // fpgR):
                    ps = ppool1.tile([P, fpgR * capR], mybir.dt.float32, tag="ps1")
                    pairs[fg] = ps
                    for off in range(fpgR):
                        f = fg * fpgR + off
                        for k in range(KT):
                            nc.tensor.matmul(
                                ps[:, off * capR : (off + 1) * capR],
                                w1_sb[s][:, k * H + f * P : k * H + (f + 1) * P],
                                xr_sb[:, k * GR + s * capR : k * GR + s * capR + capR],
                                start=(k == 0),
                                stop=(k == KT - 1),
                            )
                hl = []
                for f in range(KT):
                    fg, off = divmod(f, fpgR)
                    h1 = hpool.tile([P, capR], dt, tag=f"h1_{f}")
                    nc.scalar.activation(
                        h1[:],
                        pairs[fg][:, off * capR : (off + 1) * capR],
                        GELU,
                        bias=b1_sb[:, s * KT + f : s * KT + f + 1],
                    )
                    hl.append(h1)
                h1_tiles[s] = hl

            def emit_mm2(s):
                o = opool.tile([P, KT * capR], OUT_DT, tag="yr_o")
                for hg in range(KT // fpgR):
                    ps = ppool2.tile([P, fpgR * capR], mybir.dt.float32, tag="ps2")
                    for off in range(fpgR):
                        hidx = hg * fpgR + off
                        for f in range(KT):
                            nc.tensor.matmul(
                                ps[:, off * capR : (off + 1) * capR],
                                w2_sb[s][:, f * H + hidx * P : f * H + (hidx + 1) * P],
                                h1_tiles[s][f][:],
                                start=(f == 0),
                                stop=(f == KT - 1),
                            )
                    nc.vector.tensor_copy(
                        o[:, hg * fpgR * capR : (hg + 1) * fpgR * capR], ps[:]
                    )
                nc.scalar.dma_start(
                    out=yrp[:, s * KT * capR : (s + 1) * KT * capR], in_=o[:]
                )

            emit_mm1(0)
            for s in range(1, S):
                emit_mm1(s)
                emit_mm2(s - 1)
            emit_mm2(S - 1)

    nc.compile()
    return nc


def _sigmoid(v):
    out = np.empty_like(v)
    np.negative(np.abs(v), out=out)
    np.exp(out, out=out)
    pos = v >= 0
    out_pos = 1.0 / (1.0 + out)
    out_neg = out / (1.0 + out)
    return np.where(pos, out_pos, out_neg)


def _pack_weight(w, np_dt):
    """[S, R*P, C] -> SBUF image [P, S*R*C] (R = rows/P k-tiles)."""
    s, rp, c = w.shape
    r = rp // P
    return np.ascontiguousarray(
        w.reshape(s, r, P, c).transpose(2, 0, 1, 3).reshape(P, s * r * c)
    ).astype(np_dt)


def _pack_acts(xt, np_dt):
    """[H, G] -> SBUF image [P, KT*G]."""
    g = xt.shape[1]
    return np.ascontiguousarray(
        xt.reshape(KT, P, g).transpose(1, 0, 2).reshape(P, KT * g)
    ).astype(np_dt)


def kernel(**inputs) -> np.ndarray:
    x = np.ascontiguousarray(np.asarray(inputs["x"], dtype=np.float32))
    shared_W1 = np.asarray(inputs["shared_W1"], dtype=np.float32)
    shared_b1 = np.asarray(inputs["shared_b1"], dtype=np.float32)
    shared_W2 = np.asarray(inputs["shared_W2"], dtype=np.float32)
    shared_b2 = np.asarray(inputs["shared_b2"], dtype=np.float32)
    shared_router_W = np.asarray(inputs["shared_router_W"], dtype=np.float32)
    shared_router_b = np.asarray(inputs["shared_router_b"], dtype=np.float32)
    expert_W1 = np.asarray(inputs["expert_W1"], dtype=np.float32)
    expert_b1 = np.asarray(inputs["expert_b1"], dtype=np.float32)
    expert_W2 = np.asarray(inputs["expert_W2"], dtype=np.float32)
    expert_b2 = np.asarray(inputs["expert_b2"], dtype=np.float32)
    router_W = np.asarray(inputs["router_W"], dtype=np.float32)
    router_b = np.asarray(inputs["router_b"], dtype=np.float32)
    sub_router_W = np.asarray(inputs["sub_router_W"], dtype=np.float32)
    sub_router_b = np.asarray(inputs["sub_router_b"], dtype=np.float32)
    expert_bias = np.asarray(inputs["expert_bias"], dtype=np.float32)
    sub_expert_bias = np.asarray(inputs["sub_expert_bias"], dtype=np.float32)

    n = x.shape[0]
    assert x.shape == (N, H)

    # ---- host routing (matches reference's router math) --------------------
    sp = _sigmoid(x @ shared_router_W + shared_router_b + sub_expert_bias)  # [n,S]
    si = np.argmax(sp, axis=1)  # top-1 shared sub-expert
    sw = sp[np.arange(n), si]

    rp = _sigmoid(x @ router_W + router_b + expert_bias)  # [n,E]
    ei = np.argsort(-rp, axis=1, kind="stable")[:, :2]  # top-2 experts
    ew = np.take_along_axis(rp, ei, axis=1)  # [n,2]

    subp = _sigmoid(x @ sub_router_W + sub_router_b + sub_expert_bias)
    ssi = np.argmax(subp, axis=1)  # top-1 routed sub-expert (gate NOT applied)

    # ---- dispatch: group routed slots by (expert, sub-expert) --------------
    flat_tok = np.repeat(np.arange(n), 2)
    flat_e = ei.reshape(-1)
    flat_gate = ew.reshape(-1)
    flat_s = ssi[flat_tok]
    group = flat_e * S + flat_s
    counts = np.bincount(group, minlength=E * S)
    capR = max(64, _roundup(int(counts.max()), 16))

    sort_idx = np.argsort(group, kind="stable")
    g_tok = flat_tok[sort_idx]
    g_gate = flat_gate[sort_idx]
    g_off = np.concatenate([[0], np.cumsum(counts)])

    # shared groups: per core slice of 256 tokens, grouped by si
    capS_counts = []
    for c in range(NCORES):
        sl = si[c * N_PER_CORE : (c + 1) * N_PER_CORE]
        capS_counts.append(np.bincount(sl, minlength=S))
    capS_counts = np.stack(capS_counts)  # [NCORES, S]
    capS = max(32, _roundup(int(capS_counts.max()), 16))

    np_dt = _np_dt(MM_DT)
    xT = np.ascontiguousarray(x.T)  # [H, N] fp32; cast after gather

    GR, GS = S * capR, S * capS
    in_maps = []
    tok_es = {}
    stok_cs = {}
    for c in range(NCORES):
        e = c
        xr_host = np.zeros((H, GR), dtype=np.float32)
        for s in range(S):
            g = e * S + s
            toks = g_tok[g_off[g] : g_off[g + 1]]
            tok_es[e, s] = (toks, g_gate[g_off[g] : g_off[g + 1]])
            xr_host[:, s * capR : s * capR + len(toks)] = xT[:, toks]

        xs_host = np.zeros((H, GS), dtype=np.float32)
        base = c * N_PER_CORE
        sl = si[base : base + N_PER_CORE]
        for s in range(S):
            toks = base + np.nonzero(sl == s)[0]
            stok_cs[c, s] = toks
            xs_host[:, s * capS : s * capS + len(toks)] = xT[:, toks]

        b1p = np.ascontiguousarray(
            expert_b1[e].reshape(S, KT, P).transpose(2, 0, 1).reshape(P, S * KT)
        ).astype(np.float32)
        sb1p = np.ascontiguousarray(shared_b1.T).astype(np.float32)

        in_maps.append(
            {
                "xrp": _pack_acts(xr_host, np_dt),
                "xsp": _pack_acts(xs_host, np_dt),
                "w1p": _pack_weight(expert_W1[e], np_dt),
                "w2p": _pack_weight(expert_W2[e], np_dt),
                "sw1p": _pack_weight(shared_W1, np_dt),
                "sw2p": _pack_weight(shared_W2, np_dt),
                "b1p": b1p,
                "sb1p": sb1p,
            }
        )

    key = (capR, capS, MM_DT)
    nc = _compiled_cache.get(key)
    if nc is None:
        import time as _time

        _t = _time.time()
        nc = _build_module(capR, capS)
        print(f"[kernel] built module capR={capR} capS={capS} "
              f"in {_time.time() - _t:.1f}s", flush=True)
        _compiled_cache[key] = nc

    res = run_bass_kernel_spmd(nc, in_maps, core_ids=list(range(NCORES)))
    global _last_res
    _last_res = res

    # ---- host combine ------------------------------------------------------
    out = np.zeros((N, H), dtype=np.float32)
    for c in range(NCORES):
        e = c
        # unpack SBUF images: [P, S*KT*cap] -> per (s): [H, cap]
        yr_out = (
            res.results[c]["yrp"].reshape(P, S, KT, capR).transpose(1, 2, 0, 3)
        ).reshape(S, H, capR)
        ys_out = (
            res.results[c]["ysp"].reshape(P, S, KT, capS).transpose(1, 2, 0, 3)
        ).reshape(S, H, capS)
        for s in range(S):
            toks, gates = tok_es[e, s]
            if len(toks):
                ycols = yr_out[s, :, : len(toks)].T.astype(np.float32)  # [cnt, H]
                out[toks] += gates[:, None] * (ycols + expert_b2[e, s])
            stoks = stok_cs[c, s]
            if len(stoks):
                ycols = ys_out[s, :, : len(stoks)].T.astype(np.float32)
                out[stoks] += sw[stoks, None] * (ycols + shared_b2[s])

    return out


# revision 22
# speedup vs baseline: 1.2694x; 1.1736x over previous
"""DeepSeekMoE Trainium2 kernel: expert-parallel across 8 NeuronCores.

Strategy:
  - Host computes routing (3 small sigmoid routers + top-k) and performs the
    token all-to-all: for each expert e (= core e), gather the tokens that
    chose e in their top-2, grouped by their top-1 sub-expert, padded to a
    static capacity.  Tokens ship transposed ([H, tokens]) so the two matmul
    chains need no on-chip transposes:
        H1^T = W1^T @ X^T   (gelu + b1 fused on ScalarE)
        Y^T  = W2^T @ H1^T
    Weights are the stationary PE operand in their natural [in, out] layout.
  - The shared-expert path is data-parallel: core c processes tokens
    [c*256, (c+1)*256), grouped by top-1 shared sub-expert.  It runs FIRST
    (its weights are small) to warm the PE while expert weights stream in.
  - Every device tensor is host-prepacked into its SBUF image ([128 x free],
    contiguous per partition) so each load/store is one large-chunk 2D DMA:
    SWDGE descriptor-generation cost (~0.6-1.7us per fragmented dma_start)
    was the dominant serial cost otherwise.
  - Host applies the second bias + sigmoid gates and scatter-adds back.
Only the routing/dispatch (<1% of FLOPs) runs on host; both FFN chains for
every selected (expert, sub-expert) combo run on device in fp16 (PE runs
fp16 at 4x the fp32 rate; fp32 PSUM accumulation keeps rel-err ~3e-4).
"""

import sys

sys.path.insert(0, "/opt/trn_rl_repo")

import numpy as np

import concourse.bass as bass  # noqa: F401  (registers AP machinery)
import concourse.mybir as mybir
from concourse import bacc
from concourse.tile import TileContext
from concourse.bass_utils import run_bass_kernel_spmd

# If tracing is requested (BASS_TRACE=1) bass_utils imports
# antenv.axon_hooks, which this image's antenv package lacks — install a
# shim wired to the ctypes NTFF hook so tracing degrades gracefully.
try:
    import antenv.axon_hooks  # noqa: F401
except ImportError:
    try:
        import types as _types

        import antenv as _antenv

        _hooks = _types.ModuleType("antenv.axon_hooks")
        _hook_box = [None]
        _hooks.set_axon_ntff_profile_hook = lambda h: _hook_box.__setitem__(0, h)
        _hooks.get_axon_ntff_profile_hook = lambda: _hook_box[0]
        sys.modules["antenv.axon_hooks"] = _hooks
        _antenv.axon_hooks = _hooks
        try:
            from trn_agent_boot.trn_boot import _ntff_profile_via_ctypes

            _hooks.set_axon_ntff_profile_hook(
                _ntff_profile_via_ctypes("/opt/axon/libaxon_pjrt.so")
            )
        except Exception:
            pass
    except Exception:
        pass

N, H, E, S = 2048, 512, 8, 4
F_SH = 128
P = 128
KT = H // P  # 4 k-tiles over the hidden dim
NCORES = 8
N_PER_CORE = N // NCORES

MM_DT = mybir.dt.float16
OUT_DT = mybir.dt.float16

_compiled_cache: dict = {}
_last_res = None


def _roundup(v: int, m: int) -> int:
    return ((v + m - 1) // m) * m


def _np_dt(dt):
    return mybir.dt.np(dt)


def _build_module(capR: int, capS: int):
    """Build + compile the SPMD per-core Bass module."""
    GR = S * capR
    GS = S * capS

    nc = bacc.Bacc("TRN2", target_bir_lowering=False, debug=False)
    dt = MM_DT

    # All inputs/outputs are SBUF images: [128, free], contiguous.
    SHW = S * KT * F_SH + S * H  # sw1 ++ sw2 merged
    xrp = nc.dram_tensor("xrp", [P, KT * GR], dt, kind="ExternalInput").ap()
    xsp = nc.dram_tensor("xsp", [P, KT * GS], dt, kind="ExternalInput").ap()
    w1p = nc.dram_tensor("w1p", [P, S * KT * H], dt, kind="ExternalInput").ap()
    w2p = nc.dram_tensor("w2p", [P, S * KT * H], dt, kind="ExternalInput").ap()
    shwp = nc.dram_tensor("shwp", [P, SHW], dt, kind="ExternalInput").ap()
    # biases merged: [sb1 (S) ++ b1 (S*KT)] fp32
    bp = nc.dram_tensor("bp", [P, S + S * KT], mybir.dt.float32, kind="ExternalInput").ap()

    yrp = nc.dram_tensor("yrp", [P, S * KT * capR], OUT_DT, kind="ExternalOutput").ap()
    ysp = nc.dram_tensor("ysp", [P, S * KT * capS], OUT_DT, kind="ExternalOutput").ap()

    GELU = mybir.ActivationFunctionType.Gelu
    # pair two f/h tiles per PSUM bank when they fit (halves bank pressure
    # so two sub-expert groups can be in flight)
    fpgR = 2 if 2 * capR * 4 <= 2048 else 1
    fpgS = 2 if 2 * capS * 4 <= 2048 else 1

    with TileContext(nc) as tc:
        with (
            tc.tile_pool(name="weights", bufs=1) as wpool,
            tc.tile_pool(name="acts", bufs=1) as apool,
            tc.tile_pool(name="h1s", bufs=2) as hpool,
            tc.tile_pool(name="outs", bufs=2) as opool,
            tc.tile_pool(name="psum1", bufs=4, space="PSUM") as ppool1,
            tc.tile_pool(name="psum2", bufs=3, space="PSUM") as ppool2,
            tc.tile_pool(name="psumw", bufs=1, space="PSUM") as ppoolw,
        ):
            # ---- loads, split across BOTH HWDGE queues (sync + scalar) in
            # PE-consumption order; each dma_start costs ~600ns of sequencer
            # time, so two parallel issue streams halve the serial issue span.
            xs_sb = apool.tile([P, KT * GS], dt, tag="xs")
            nc.sync.dma_start(out=xs_sb[:], in_=xsp[:])
            shw_sb = wpool.tile([P, SHW], dt, tag="shw")
            nc.sync.dma_start(out=shw_sb[:], in_=shwp[:])
            sw1_sb = shw_sb[:, : S * KT * F_SH]
            sw2_sb = shw_sb[:, S * KT * F_SH :]
            b_sb = wpool.tile([P, S + S * KT], mybir.dt.float32, tag="bp")
            nc.scalar.dma_start(out=b_sb[:], in_=bp[:])
            sb1_sb = b_sb[:, :S]
            b1_sb = b_sb[:, S:]
            xr_sb = apool.tile([P, KT * GR], dt, tag="xr")
            nc.sync.dma_start(out=xr_sb[:], in_=xrp[:])
            w1_sb = {}
            w2_sb = {}
            for s in range(S):
                w1_sb[s] = wpool.tile([P, KT * H], dt, tag=f"w1_{s}", name=f"w1_{s}")
                w2_sb[s] = wpool.tile([P, KT * H], dt, tag=f"w2_{s}", name=f"w2_{s}")
            # issue in consumption order of the software pipeline:
            #   w1[0], w1[1], w2[0], w1[2], w2[1], w1[3], w2[2], w2[3]
            worder = [(w1_sb, 0), (w1_sb, 1), (w2_sb, 0), (w1_sb, 2),
                      (w2_sb, 1), (w1_sb, 3), (w2_sb, 2), (w2_sb, 3)]
            for i, (tbl, s) in enumerate(worder):
                src = w1p if tbl is w1_sb else w2p
                eng = nc.sync if i % 2 == 0 else nc.scalar
                eng.dma_start(
                    out=tbl[s][:], in_=src[:, s * KT * H : (s + 1) * KT * H]
                )

            # ---- PE warm-up: ~3.5us of dummy matmuls during the DMA wait
            # flips the HAM clock gate to 2.4GHz before the real work lands.
            zw = wpool.tile([P, 512], dt, tag="warm_in")
            nc.vector.memset(zw[:], 0.0)
            wps = ppoolw.tile([P, 512], mybir.dt.float32, tag="warm_ps")
            for _ in range(8):
                nc.tensor.matmul(wps, zw[:, :P], zw[:], start=True, stop=True)
            # dummy activation: pulls the ~1.3us Gelu LUT load off the
            # critical path (it happens during the DMA wait instead)
            zg = apool.tile([P, 1], dt, tag="warm_gelu")
            nc.scalar.activation(zg[:], zw[:, :1], GELU, bias=0.0)

            # ---- shared path (small; runs first while weights stream in) ---
            sh_ps1 = {}
            for s in range(S):
                g, off = divmod(s, fpgS)
                if off == 0:
                    sh_ps1[g] = ppool1.tile(
                        [P, fpgS * capS], mybir.dt.float32, tag="ps1", name=f"shps1_{g}"
                    )
                for k in range(KT):
                    nc.tensor.matmul(
                        sh_ps1[g][:, off * capS : (off + 1) * capS],
                        sw1_sb[:, (s * KT + k) * F_SH : (s * KT + k + 1) * F_SH],
                        xs_sb[:, k * GS + s * capS : k * GS + s * capS + capS],
                        start=(k == 0),
                        stop=(k == KT - 1),
                    )
            hs_sb = {}
            for s in range(S):
                g, off = divmod(s, fpgS)
                hs = hpool.tile([P, capS], dt, tag=f"hs_{s}")
                nc.scalar.activation(
                    hs[:],
                    sh_ps1[g][:, off * capS : (off + 1) * capS],
                    GELU,
                    bias=sb1_sb[:, s : s + 1],
                )
                hs_sb[s] = hs
            for s in range(S):
                o = opool.tile([P, KT * capS], OUT_DT, tag="ys_o")
                for hg in range(KT // fpgS):
                    ps2 = ppool2.tile([P, fpgS * capS], mybir.dt.float32, tag="ps2")
                    for off in range(fpgS):
                        hidx = hg * fpgS + off
                        nc.tensor.matmul(
                            ps2[:, off * capS : (off + 1) * capS],
                            sw2_sb[:, s * H + hidx * P : s * H + (hidx + 1) * P],
                            hs_sb[s][:],
                            start=True,
                            stop=True,
                        )
                    nc.vector.tensor_copy(
                        o[:, hg * fpgS * capS : (hg + 1) * fpgS * capS], ps2[:]
                    )
                nc.gpsimd.dma_start(
                    out=ysp[:, s * KT * capS : (s + 1) * KT * capS], in_=o[:]
                )

            # ---- routed path, software-pipelined on the PE:
            #   MM1(0), MM1(1), MM2(0), MM1(2), MM2(1), MM1(3), MM2(2), MM2(3)
            # so the gelu latency between a sub-expert's two chains is hidden
            # behind the next sub-expert's first chain.
            h1_tiles = {}

            def emit_mm1(s):
                pairs = {}
                for fg in range(KT // fpgR):
                    ps = ppool1.tile([P, fpgR * capR], mybir.dt.float32, tag="ps1")
                    pairs[fg] = ps
                    for off in range(fpgR):
                        f = fg * fpgR + off
                        for k in range(KT):
                            nc.tensor.matmul(
                                ps[:, off * capR : (off + 1) * capR],
                                w1_sb[s][:, k * H + f * P : k * H + (f + 1) * P],
                                xr_sb[:, k * GR + s * capR : k * GR + s * capR + capR],
                                start=(k == 0),
                                stop=(k == KT - 1),
                            )
                hl = []
                for f in range(KT):
                    fg, off = divmod(f, fpgR)
                    h1 = hpool.tile([P, capR], dt, tag=f"h1_{f}")
                    nc.scalar.activation(
                        h1[:],
                        pairs[fg][:, off * capR : (off + 1) * capR],
                        GELU,
                        bias=b1_sb[:, s * KT + f : s * KT + f + 1],
                    )
                    hl.append(h1)
                h1_tiles[s] = hl

            def emit_mm2(s):
                o = opool.tile([P, KT * capR], OUT_DT, tag="yr_o")
                for hg in range(KT // fpgR):
                    ps = ppool2.tile([P, fpgR * capR], mybir.dt.float32, tag="ps2")
                    for off in range(fpgR):
                        hidx = hg * fpgR + off
                        for f in range(KT):
                            nc.tensor.matmul(
                                ps[:, off * capR : (off + 1) * capR],
                                w2_sb[s][:, f * H + hidx * P : f * H + (hidx + 1) * P],
                                h1_tiles[s][f][:],
                                start=(f == 0),
                                stop=(f == KT - 1),
                            )
                    nc.vector.tensor_copy(
                        o[:, hg * fpgR * capR : (hg + 1) * fpgR * capR], ps[:]
                    )
                nc.scalar.dma_start(
                    out=yrp[:, s * KT * capR : (s + 1) * KT * capR], in_=o[:]
                )

            emit_mm1(0)
            for s in range(1, S):
                emit_mm1(s)
                emit_mm2(s - 1)
            emit_mm2(S - 1)

    nc.compile()
    return nc


def _sigmoid(v):
    out = np.empty_like(v)
    np.negative(np.abs(v), out=out)
    np.exp(out, out=out)
    pos = v >= 0
    out_pos = 1.0 / (1.0 + out)
    out_neg = out / (1.0 + out)
    return np.where(pos, out_pos, out_neg)


def _pack_weight(w, np_dt):
    """[S, R*P, C] -> SBUF image [P, S*R*C] (R = rows/P k-tiles)."""
    s, rp, c = w.shape
    r = rp // P
    return np.ascontiguousarray(
        w.reshape(s, r, P, c).transpose(2, 0, 1, 3).reshape(P, s * r * c)
    ).astype(np_dt)


def _pack_acts(xt, np_dt):
    """[H, G] -> SBUF image [P, KT*G]."""
    g = xt.shape[1]
    return np.ascontiguousarray(
        xt.reshape(KT, P, g).transpose(1, 0, 2).reshape(P, KT * g)
    ).astype(np_dt)


def kernel(**inputs) -> np.ndarray:
    x = np.ascontiguousarray(np.asarray(inputs["x"], dtype=np.float32))
    shared_W1 = np.asarray(inputs["shared_W1"], dtype=np.float32)
    shared_b1 = np.asarray(inputs["shared_b1"], dtype=np.float32)
    shared_W2 = np.asarray(inputs["shared_W2"], dtype=np.float32)
    shared_b2 = np.asarray(inputs["shared_b2"], dtype=np.float32)
    shared_router_W = np.asarray(inputs["shared_router_W"], dtype=np.float32)
    shared_router_b = np.asarray(inputs["shared_router_b"], dtype=np.float32)
    expert_W1 = np.asarray(inputs["expert_W1"], dtype=np.float32)
    expert_b1 = np.asarray(inputs["expert_b1"], dtype=np.float32)
    expert_W2 = np.asarray(inputs["expert_W2"], dtype=np.float32)
    expert_b2 = np.asarray(inputs["expert_b2"], dtype=np.float32)
    router_W = np.asarray(inputs["router_W"], dtype=np.float32)
    router_b = np.asarray(inputs["router_b"], dtype=np.float32)
    sub_router_W = np.asarray(inputs["sub_router_W"], dtype=np.float32)
    sub_router_b = np.asarray(inputs["sub_router_b"], dtype=np.float32)
    expert_bias = np.asarray(inputs["expert_bias"], dtype=np.float32)
    sub_expert_bias = np.asarray(inputs["sub_expert_bias"], dtype=np.float32)

    n = x.shape[0]
    assert x.shape == (N, H)

    # ---- host routing (matches reference's router math) --------------------
    sp = _sigmoid(x @ shared_router_W + shared_router_b + sub_expert_bias)  # [n,S]
    si = np.argmax(sp, axis=1)  # top-1 shared sub-expert
    sw = sp[np.arange(n), si]

    rp = _sigmoid(x @ router_W + router_b + expert_bias)  # [n,E]
    ei = np.argsort(-rp, axis=1, kind="stable")[:, :2]  # top-2 experts
    ew = np.take_along_axis(rp, ei, axis=1)  # [n,2]

    subp = _sigmoid(x @ sub_router_W + sub_router_b + sub_expert_bias)
    ssi = np.argmax(subp, axis=1)  # top-1 routed sub-expert (gate NOT applied)

    # ---- dispatch: group routed slots by (expert, sub-expert) --------------
    flat_tok = np.repeat(np.arange(n), 2)
    flat_e = ei.reshape(-1)
    flat_gate = ew.reshape(-1)
    flat_s = ssi[flat_tok]
    group = flat_e * S + flat_s
    counts = np.bincount(group, minlength=E * S)
    capR = max(64, _roundup(int(counts.max()), 16))

    sort_idx = np.argsort(group, kind="stable")
    g_tok = flat_tok[sort_idx]
    g_gate = flat_gate[sort_idx]
    g_off = np.concatenate([[0], np.cumsum(counts)])

    # shared groups: per core slice of 256 tokens, grouped by si
    capS_counts = []
    for c in range(NCORES):
        sl = si[c * N_PER_CORE : (c + 1) * N_PER_CORE]
        capS_counts.append(np.bincount(sl, minlength=S))
    capS_counts = np.stack(capS_counts)  # [NCORES, S]
    capS = max(32, _roundup(int(capS_counts.max()), 16))

    np_dt = _np_dt(MM_DT)
    xT = np.ascontiguousarray(x.T)  # [H, N] fp32; cast after gather

    GR, GS = S * capR, S * capS
    in_maps = []
    tok_es = {}
    stok_cs = {}
    for c in range(NCORES):
        e = c
        xr_host = np.zeros((H, GR), dtype=np.float32)
        for s in range(S):
            g = e * S + s
            toks = g_tok[g_off[g] : g_off[g + 1]]
            tok_es[e, s] = (toks, g_gate[g_off[g] : g_off[g + 1]])
            xr_host[:, s * capR : s * capR + len(toks)] = xT[:, toks]

        xs_host = np.zeros((H, GS), dtype=np.float32)
        base = c * N_PER_CORE
        sl = si[base : base + N_PER_CORE]
        for s in range(S):
            toks = base + np.nonzero(sl == s)[0]
            stok_cs[c, s] = toks
            xs_host[:, s * capS : s * capS + len(toks)] = xT[:, toks]

        b1p = np.ascontiguousarray(
            expert_b1[e].reshape(S, KT, P).transpose(2, 0, 1).reshape(P, S * KT)
        ).astype(np.float32)
        sb1p = np.ascontiguousarray(shared_b1.T).astype(np.float32)

        in_maps.append(
            {
                "xrp": _pack_acts(xr_host, np_dt),
                "xsp": _pack_acts(xs_host, np_dt),
                "w1p": _pack_weight(expert_W1[e], np_dt),
                "w2p": _pack_weight(expert_W2[e], np_dt),
                "shwp": np.ascontiguousarray(
                    np.concatenate(
                        [_pack_weight(shared_W1, np_dt), _pack_weight(shared_W2, np_dt)],
                        axis=1,
                    )
                ),
                "bp": np.ascontiguousarray(np.concatenate([sb1p, b1p], axis=1)),
            }
        )

    key = (capR, capS, MM_DT)
    nc = _compiled_cache.get(key)
    if nc is None:
        import time as _time

        _t = _time.time()
        nc = _build_module(capR, capS)
        print(f"[kernel] built module capR={capR} capS={capS} "
              f"in {_time.time() - _t:.1f}s", flush=True)
        _compiled_cache[key] = nc

    res = run_bass_kernel_spmd(nc, in_maps, core_ids=list(range(NCORES)))
    global _last_res
    _last_res = res

    # ---- host combine ------------------------------------------------------
    out = np.zeros((N, H), dtype=np.float32)
    for c in range(NCORES):
        e = c
        # unpack SBUF images: [P, S*KT*cap] -> per (s): [H, cap]
        yr_out = (
            res.results[c]["yrp"].reshape(P, S, KT, capR).transpose(1, 2, 0, 3)
        ).reshape(S, H, capR)
        ys_out = (
            res.results[c]["ysp"].reshape(P, S, KT, capS).transpose(1, 2, 0, 3)
        ).reshape(S, H, capS)
        for s in range(S):
            toks, gates = tok_es[e, s]
            if len(toks):
                ycols = yr_out[s, :, : len(toks)].T.astype(np.float32)  # [cnt, H]
                out[toks] += gates[:, None] * (ycols + expert_b2[e, s])
            stoks = stok_cs[c, s]
            if len(stoks):
                ycols = ys_out[s, :, : len(stoks)].T.astype(np.float32)
                out[stoks] += sw[stoks, None] * (ycols + shared_b2[s])

    return out
